# revision 20
# baseline (speedup 1.0000x reference)
"""Chamfer distance kernel for Trainium2 (8 NeuronCores, Bass/Tile).

Strategy
--------
dist2[b, i, j] = ||targets[b,i] - preds[b,j]||^2 is computed on the tensor
engine with a K=9 "homogeneous coordinate" encoding:

    d2 = sum_d (t_d^2 * 1  +  t_d * (-2 p_d)  +  1 * p_d^2)

so a single matmul with contraction K=9 produces squared distances directly
in PSUM (fp32, N=512 per bank; 4 matmuls fill a 4-bank [128, 2048] tile).
VectorE then consumes each PSUM tile exactly twice, straight from PSUM:
  * row-min:  one tensor_reduce(min) per tile -> per-(i-tile, j-half) partial
  * col-min:  one tensor_tensor(min) accumulate into an fp16 column
              accumulator (fp32 PSUM in, fp16 out)
The 128-partition fold of the column accumulator is one blocked xbar DMA
transpose (out[p,k,q] = in[q, k*128+p]) plus one batched reduce.
sqrt + means + cross-core combining happen on the host (O(N) work only; all
O(N^2) work stays on device).

Sharding: 8 cores = 4 batches x 2 target-halves.  Each core computes its
2048 x 4096 block of the distance matrix: row-mins are complete per core;
col-mins are partial (its target half) and the two halves are min-combined
on the host.

This environment's axon backend executes Bass NEFFs at a ~55-60us
per-instruction floor (emulated NRT), so the kernel minimizes instruction
count: 128 matmuls (the hard floor: fp32 PSUM output is capped at 512
columns/bank on TRN2) + 64 VectorE ops + 4 finale/IO ops per body.
"""

import sys

sys.path.insert(0, "/opt/trn_rl_repo")

import numpy as np

import concourse.bass as bass
import concourse.bacc as bacc
import concourse.tile as tile
from concourse import mybir

B, N, D = 4, 4096, 3
NCORES = 8
HALF = N // 2          # targets per core
NIT = HALF // 128      # 16 i-tiles of 128 rows
NTB = N // 128         # 32 col-fold blocks of 128 preds
K = 9                  # homogeneous encoding dim

F32 = mybir.dt.float32
F16 = mybir.dt.float16
BIG = 60000.0          # min-accumulator init (fits fp16; > any d2 here)


def _chamfer_tile_kernel(tc, rowmin, colmin, tpq, repeat=1):
    from contextlib import ExitStack

    nc = tc.nc
    MN = mybir.AluOpType.min

    with ExitStack() as ctx:
        consts = ctx.enter_context(tc.tile_pool(name="consts", bufs=1))
        accs = ctx.enter_context(tc.tile_pool(name="accs", bufs=1))
        psums = ctx.enter_context(tc.tile_pool(name="psums", bufs=2, space="PSUM"))
        outsp = ctx.enter_context(tc.tile_pool(name="outsp", bufs=1))

        # packed operands: [:, :HALF] = targets enc, [:, HALF:] = preds enc
        tpq_s = consts.tile([K, HALF + N], F32, tag="tpq")
        nc.sync.dma_start(out=tpq_s[:], in_=tpq)
        tq_s = tpq_s[:, :HALF]
        pq_s = tpq_s[:, HALF:]

        rowmin_s = outsp.tile([128, NIT], F32, tag="rowmin")
        colmin_s = outsp.tile([128, NTB], F32, tag="colmin")

        for _rep in range(repeat):   # repeat>1 is used only for timing
            _emit_body(tc, accs, psums, tq_s, pq_s, rowmin_s, colmin_s, MN)

        nc.sync.dma_start(out=rowmin, in_=rowmin_s[:])
        nc.sync.dma_start(out=colmin, in_=colmin_s[:])


def _emit_body(tc, accs, psums, tq_s, pq_s, rowmin_s, colmin_s, MN):
    nc = tc.nc
    rowparts = accs.tile([128, NIT, 2], F32, tag="rowparts")
    colacc = accs.tile([128, 2, 2048], F16, tag="colacc")
    colaccT = accs.tile([128, 2, 16, 128], F16, tag="colaccT")
    nc.vector.memset(colacc[:], BIG)

    for jo in range(2):          # j-half: preds [jo*2048, (jo+1)*2048)
        for it in range(NIT):
            ps = psums.tile([128, 2048], F32, tag="ps")
            for jtl in range(4):
                j0 = jo * 2048 + jtl * 512
                nc.tensor.matmul(
                    ps[:, jtl * 512:(jtl + 1) * 512],
                    tq_s[:, it * 128:(it + 1) * 128],
                    pq_s[:, j0:j0 + 512],
                    start=True,
                    stop=True,
                )
            # row-min of this tile (over its 2048 j's) straight from PSUM
            nc.vector.tensor_reduce(
                rowparts[:, it, jo:jo + 1],
                ps[:],
                axis=mybir.AxisListType.X,
                op=MN,
            )
            # col-min accumulate straight from PSUM (fp32 in -> fp16 acc)
            nc.vector.tensor_tensor(
                colacc[:, jo, :], colacc[:, jo, :], ps[:], MN
            )
    # fold col-min over the 128 partitions: one blocked xbar transpose
    # (out[p, k, q] = colacc[q, k*128+p]) + one batched reduce
    nc.sync.dma_start_transpose(
        colaccT[:].rearrange("p a b f -> p (a b) f"),
        colacc[:].rearrange("p a b -> p (a b)"),
    )
    nc.vector.tensor_reduce(
        colmin_s[:],
        colaccT[:],
        axis=mybir.AxisListType.X,
        op=MN,
    )
    nc.vector.tensor_reduce(
        rowmin_s[:],
        rowparts[:],
        axis=mybir.AxisListType.X,
        op=MN,
    )


_PROGRAMS = {}


def build_program(repeat=1):
    if repeat in _PROGRAMS:
        return _PROGRAMS[repeat]
    nc = bacc.Bacc("TRN2", target_bir_lowering=False, debug=False,
                   num_devices=NCORES)
    tpq = nc.dram_tensor("tpq", [K, HALF + N], F32, kind="ExternalInput").ap()
    rowmin = nc.dram_tensor("rowmin", [128, NIT], F32, kind="ExternalOutput").ap()
    colmin = nc.dram_tensor("colmin", [128, NTB], F32, kind="ExternalOutput").ap()
    with tile.TileContext(nc) as tc:
        _chamfer_tile_kernel(tc, rowmin, colmin, tpq, repeat=repeat)
    nc.compile()   # Bacc passes split multi-waits off matmuls (walrus limit)
    _PROGRAMS[repeat] = nc
    return nc


def make_in_maps(preds, targets):
    """Host-side shard + encode (O(N) prep only)."""
    preds = np.asarray(preds, dtype=np.float32)
    targets = np.asarray(targets, dtype=np.float32)
    in_maps = []
    for c in range(NCORES):
        b, h = divmod(c, 2)
        t = targets[b, h * HALF:(h + 1) * HALF]   # (2048, 3)
        p = preds[b]                              # (4096, 3)
        tpq = np.empty((K, HALF + N), np.float32)
        for d in range(D):
            tpq[3 * d + 0, :HALF] = t[:, d] * t[:, d]
            tpq[3 * d + 1, :HALF] = t[:, d]
            tpq[3 * d + 2, :HALF] = 1.0
            tpq[3 * d + 0, HALF:] = 1.0
            tpq[3 * d + 1, HALF:] = -2.0 * p[:, d]
            tpq[3 * d + 2, HALF:] = p[:, d] * p[:, d]
        in_maps.append({"tpq": tpq})
    return in_maps


def unshard(results):
    """Combine per-core row/col minima -> chamfer scalar (host, O(N))."""
    row_sqrts = []
    col_halves = []
    for c in range(NCORES):
        rm = np.asarray(results[c]["rowmin"], np.float32).T.reshape(HALF)
        cm = np.asarray(results[c]["colmin"], np.float32).T.reshape(N)
        row_sqrts.append(np.sqrt(np.maximum(rm, 0.0)))
        col_halves.append(cm)
    row_all = np.concatenate(row_sqrts)           # 8 * 2048 = B*N target mins
    col_sqrts = []
    for b in range(B):
        cm = np.minimum(col_halves[2 * b], col_halves[2 * b + 1])
        col_sqrts.append(np.sqrt(np.maximum(cm, 0.0)))
    col_all = np.concatenate(col_sqrts)           # B*N pred mins
    return np.float32(row_all.mean() + col_all.mean())


def run(preds, targets, trace=False, **kw):
    from concourse.bass_utils import run_bass_kernel_spmd

    nc = build_program()
    in_maps = make_in_maps(preds, targets)
    res = run_bass_kernel_spmd(nc, in_maps, list(range(NCORES)), trace=trace, **kw)
    return res


def kernel(preds, targets):
    res = run(preds, targets, trace=False)
    return unshard(res.results)


if __name__ == "__main__":
    rng = np.random.default_rng(0)
    p = rng.standard_normal((B, N, D), dtype=np.float32)
    t = rng.standard_normal((B, N, D), dtype=np.float32)
    out = kernel(p, t)
    print("kernel out:", out)
